# revision 13
# baseline (speedup 1.0000x reference)
"""DeepSeek-style MoE block (SwiGLU experts, top-k routing) on 8 Trainium2 cores.

Expert-parallel: each of the 8 cores owns E/8 = 2 experts and receives only the
tokens routed to those experts (host-side dispatch).

Design (vs the fp32r baseline at ~223 us):
 - Everything streamed from HBM is bf16 (weights, gathered x, output y), which
   halves the dominant DMA traffic (56.6 MB -> ~31 MB per core) and enables the
   PE's Fast Weight Load path. Matmul rate is unchanged (1 row/cycle for bf16
   and fp32r alike); PSUM accumulates fp32. Measured end-to-end rel err of the
   bf16 pipeline is ~4.4e-3 (gate 2e-2).
 - s0/s1 are folded into w0/w1 on the host (both scales are linear
   pre-activation), s2*cw is folded into the host combine, so the device
   kernel is pure silu(x@w0T)*(x@w1T) @ w2T.
 - Tokens stay on the matmul free dim in BOTH phases (phase 2 is transposed:
   y^T = w2 @ h^T), so compute scales with the padded token count, not with
   128-tile granularity.
 - Per-slot token padding: experts are sorted by token count; the 8 largest go
   to slot 0 (TG0 = roundup8 of their max), the 8 smallest to slot 1 (TG1).
   The SPMD program bakes both shapes in, cutting ~5% of padded compute vs a
   single global TG.
 - Host pre-swizzles every DRAM operand into the exact SBUF layout so DMAs are
   large fully-contiguous transfers; expert 0's xt/w01 are chunked 4-way and
   pinned to opposite HWDGE rings so the first matmuls start ~6 us earlier.
 - Large DMAs alternate across the two HWDGE rings (nc.sync / nc.scalar); w2
   streams via SWDGE (gpsimd) so phase-2 weights don't queue behind the
   phase-1 weight stream on the rings.
"""

import os
import numpy as np
import ml_dtypes

T, D, DFF, E, TOPK = 1024, 2048, 1024, 16, 6
NCORES, P = 8, 128
EPC = E // NCORES   # experts per core
KD = D // P         # contraction tiles over D (phase 1)
KF = DFF // P       # contraction tiles over DFF (phase 2)
FG = 2              # f-tiles (128 rows of DFF) per PSUM group, phase 1
FGP = FG * P
NFG = KF // FG      # phase-1 weight groups per expert
DSW = 512           # phase-2 D-slice width
NDS = D // DSW      # phase-2 D slices
NDP = DSW // P      # d-tiles per D slice

BF16 = ml_dtypes.bfloat16

# Set by kernel() after each run: BassKernelResults (exec_time_ns when traced).
LAST_RESULT = None

_PROGRAM_CACHE = {}


def _build_program(TGs, CH):
    """SPMD single-core program. Slot shapes TGs (len EPC) are compile-time;
    CH > 1 token-chunks (uniform TG) only when some expert gets > 512 tokens;
    weights are re-streamed per chunk in that rare fallback."""
    import concourse.bacc as bacc
    import concourse.mybir as mybir
    import concourse.tile as tile

    assert all(tg <= 512 for tg in TGs)
    f32 = mybir.dt.float32
    bf16 = mybir.dt.bfloat16
    Silu = mybir.ActivationFunctionType.Silu

    V = EPC * CH
    TGM = max(TGs)
    nc = bacc.Bacc("TRN2", target_bir_lowering=False, debug=False)

    xt_d = nc.dram_tensor("xt", [V, P, KD * TGM], bf16,
                          kind="ExternalInput").ap()
    w01_d = nc.dram_tensor("w01", [EPC, NFG, P, KD * 2 * FGP], bf16,
                           kind="ExternalInput").ap()
    w2_d = nc.dram_tensor("w2", [EPC, NDS, P, KF * DSW], bf16,
                          kind="ExternalInput").ap()
    y_d = nc.dram_tensor("y", [V, NDS, P, NDP * TGM], bf16,
                         kind="ExternalOutput").ap()

    with tile.TileContext(nc) as tc:
        # Alternate large DMAs across the two physical HWDGE rings.
        rings = [nc.sync, nc.scalar]
        ring_state = [0]

        def ring():
            ring_state[0] ^= 1
            return rings[ring_state[0]]

        KCH = 4           # k-tiles per startup chunk
        NCHK = KD // KCH  # startup chunks for xt / first w01 group

        with (
            tc.tile_pool(name="xt0", bufs=NCHK) as xt0_pool,
            tc.tile_pool(name="w01", bufs=4) as w01_pool,
            tc.tile_pool(name="w010", bufs=NCHK) as w010_pool,
            tc.tile_pool(name="w2", bufs=3) as w2_pool,
            tc.tile_pool(name="ht", bufs=2) as ht_pool,
            tc.tile_pool(name="y", bufs=3) as y_pool,
            tc.tile_pool(name="ydj", bufs=4) as ydj_pool,
            tc.tile_pool(name="sig", bufs=4) as sig_pool,
            tc.tile_pool(name="pgu", bufs=6, space="PSUM") as pgu_pool,
            tc.tile_pool(name="py", bufs=2, space="PSUM") as py_pool,
        ):
            # HAM warm-up: ~4 us of tiny matmuls on a zeroed tile keep the
            # PE activity monitor busy while the first weight DMAs land, so
            # the real stream starts at 2.4 GHz instead of ramping from 1.2.
            warm = ydj_pool.tile([P, TGM], bf16, tag="ydj", name="warm_src")
            nc.vector.memset(warm[:, :64], 0.0)
            psW = pgu_pool.tile([P, TGM], f32, tag="pgu", name="warm_ps")
            for i in range(48):
                nc.tensor.matmul(psW[:64, :64], warm[:, :64], warm[:, :64],
                                 start=True, stop=True)

            for v in range(V):
                e = v // CH
                TG = TGs[e]
                # xt is always loaded in 4 k-chunks from one pool: expert 0's
                # chunks are pinned opposite the w01 chunks for a fast start,
                # and later experts' gathers queue behind (pool reuse) so
                # they stay out of the startup-critical DMA window.
                xtc = []
                for c in range(NCHK):
                    t = xt0_pool.tile([P, KCH * TGM], bf16, tag="xt0")
                    eng = nc.scalar if v == 0 else ring()
                    eng.dma_start(
                        t[:, :KCH * TG],
                        xt_d[v][:, c * KCH * TG:(c + 1) * KCH * TG])
                    xtc.append(t)

                def xt_ap(k):
                    return xtc[k // KCH][:, (k % KCH) * TG:(k % KCH + 1) * TG]

                if v == 0:
                    # First weight group in chunks, paired with xt above.
                    w01c = []
                    for c in range(NCHK):
                        t = w010_pool.tile([P, KCH, 2, FGP], bf16, tag="w010")
                        nc.sync.dma_start(
                            t[:],
                            w01_d[e, 0][:, c * KCH * 2 * FGP:
                                        (c + 1) * KCH * 2 * FGP]
                            .rearrange("p (k g c) -> p k g c", k=KCH, g=2))
                        w01c.append(t)

                # --- phase 1: ht = silu(x@w0T)*(x@w1T), [DFF, TG] bf16 ---
                ht = ht_pool.tile([P, KF * TGM], bf16, tag="ht")
                for fg in range(NFG):
                    if v == 0 and fg == 0:
                        # Chunk tiles were loaded above, paired with xt.
                        def w01_ap(k, gu, j):
                            return w01c[k // KCH][:, k % KCH, gu,
                                                  j * P:(j + 1) * P]
                    else:
                        w01b = w01_pool.tile([P, KD, 2, FGP], bf16, tag="w01b")
                        ring().dma_start(
                            w01b[:],
                            w01_d[e, fg].rearrange("p (k g c) -> p k g c",
                                                   k=KD, g=2))

                        def w01_ap(k, gu, j):
                            return w01b[:, k, gu, j * P:(j + 1) * P]
                    psG = [pgu_pool.tile([P, TGM], f32, tag="pgu",
                                         name=f"psG_{v}_{fg}_{j}")
                           for j in range(FG)]
                    psU = [pgu_pool.tile([P, TGM], f32, tag="pgu",
                                         name=f"psU_{v}_{fg}_{j}")
                           for j in range(FG)]
                    for k in range(KD):
                        for j in range(FG):
                            nc.tensor.matmul(
                                psG[j][:, :TG], w01_ap(k, 0, j), xt_ap(k),
                                start=(k == 0), stop=(k == KD - 1))
                            nc.tensor.matmul(
                                psU[j][:, :TG], w01_ap(k, 1, j), xt_ap(k),
                                start=(k == 0), stop=(k == KD - 1))
                    for j in range(FG):
                        f = fg * FG + j
                        sig = sig_pool.tile([P, TGM], f32, tag="sig")
                        nc.scalar.activation(sig[:, :TG], psG[j][:, :TG], Silu)
                        nc.vector.tensor_mul(
                            ht[:, f * TG:(f + 1) * TG], sig[:, :TG],
                            psU[j][:, :TG])

                # --- phase 2: y^T = (h @ w2T)^T = w2 @ h^T, [D, TG] bf16 ---
                for ds in range(NDS):
                    w2b = w2_pool.tile([P, KF, DSW], bf16, tag="w2b")
                    nc.gpsimd.dma_start(
                        w2b[:], w2_d[e, ds].rearrange("p (k c) -> p k c", k=KF))
                    last = (v == V - 1 and ds == NDS - 1)
                    yds = None if last else y_pool.tile([P, NDP * TGM], bf16,
                                                        tag="yds")
                    for dj in range(NDP):
                        psY = py_pool.tile([P, TGM], f32, tag="py",
                                           name=f"psY_{v}_{ds}_{dj}")
                        for k in range(KF):
                            nc.tensor.matmul(
                                psY[:, :TG], w2b[:, k, dj * P:(dj + 1) * P],
                                ht[:, k * TG:(k + 1) * TG],
                                start=(k == 0), stop=(k == KF - 1))
                        if last:
                            # Drain the final D-slice per d-tile so the last
                            # output DMA isn't one big post-compute transfer.
                            yj = ydj_pool.tile([P, TGM], bf16, tag="ydj")
                            if dj % 2 == 0:
                                nc.scalar.copy(yj[:, :TG], psY[:, :TG])
                            else:
                                nc.vector.tensor_copy(yj[:, :TG], psY[:, :TG])
                            ring().dma_start(
                                y_d[v, ds][:, dj * TG:(dj + 1) * TG],
                                yj[:, :TG])
                        elif dj % 2 == 0:
                            nc.scalar.copy(yds[:, dj * TG:(dj + 1) * TG],
                                           psY[:, :TG])
                        else:
                            nc.vector.tensor_copy(
                                yds[:, dj * TG:(dj + 1) * TG], psY[:, :TG])
                    if not last:
                        ring().dma_start(y_d[v, ds][:, :NDP * TG],
                                         yds[:, :NDP * TG])

    nc.compile()
    return nc


def _roundup8(n):
    return max(64, -(-int(n) // 8) * 8)


def _prep_host(inputs):
    """Host-side dispatch: routing weights, expert->slot assignment by token
    count, per-expert token gather, bf16 quantization, and pre-swizzle of
    every operand into its SBUF layout."""
    x = np.asarray(inputs["x"], dtype=np.float32)
    w0 = np.asarray(inputs["w0"], dtype=np.float32)
    w1 = np.asarray(inputs["w1"], dtype=np.float32)
    w2 = np.asarray(inputs["w2"], dtype=np.float32)
    s0 = np.asarray(inputs["s0"], dtype=np.float32)
    s1 = np.asarray(inputs["s1"], dtype=np.float32)
    s2 = np.asarray(inputs["s2"], dtype=np.float32)
    se = np.asarray(inputs["selected_experts"]).astype(np.int64)
    rw = np.asarray(inputs["routing_weights"], dtype=np.float32)

    Tn, Dn = x.shape
    En = w0.shape[0]
    assert (Dn, w0.shape[1], En) == (D, DFF, E)

    # combine weight per (expert, token): sum of routing weights over top-k
    cw = np.zeros((En, Tn), np.float32)
    cols = np.arange(Tn)
    for k in range(se.shape[1]):
        np.add.at(cw, (se[:, k], cols), rw[:, k])

    idx = [np.flatnonzero(cw[e] != 0.0) for e in range(En)]
    counts = np.array([len(i) for i in idx])
    maxn = int(counts.max())
    if maxn > 512:
        CH = -(-maxn // 512)
        tg = _roundup8(-(-maxn // CH))
        TGs = (tg,) * EPC
        order = np.arange(En)
    else:
        CH = 1
        # largest experts -> slot 0, smallest -> slot 1
        order = np.argsort(-counts, kind="stable")
        TGs = tuple(_roundup8(counts[order[s * NCORES:(s + 1) * NCORES]].max())
                    for s in range(EPC))
    TGM = max(TGs)

    # slot_expert[c][s] = expert id handled by core c, slot s
    slot_expert = [[int(order[s * NCORES + c]) for s in range(EPC)]
                   for c in range(NCORES)]

    xT = np.ascontiguousarray(x.T).astype(BF16)  # [D, T] bf16
    in_maps = []
    for c in range(NCORES):
        xt = np.zeros((EPC * CH, P, KD * TGM), BF16)
        w01 = np.empty((EPC, NFG, P, KD * 2 * FGP), BF16)
        w2h = np.empty((EPC, NDS, P, KF * DSW), BF16)
        for j in range(EPC):
            e = slot_expert[c][j]
            TG = TGs[j]
            ids = idx[e]
            for ch in range(CH):
                sl = ids[ch * TG:(ch + 1) * TG]
                g = np.zeros((KD, P, TG), BF16)
                g[:, :, :len(sl)] = xT[:, sl].reshape(KD, P, len(sl))
                xt[j * CH + ch, :, :KD * TG] = (
                    g.transpose(1, 0, 2).reshape(P, KD * TG))
            # [2, D, DFF] -> [NFG, P, KD, 2, FGP], scales folded in
            wt = np.stack([(w0[e] * s0[e]).T, (w1[e] * s1[e]).T], 0)
            a = wt.reshape(2, KD, P, NFG, FGP)
            w01[j] = (a.transpose(3, 2, 1, 0, 4)
                      .reshape(NFG, P, KD * 2 * FGP).astype(BF16))
            # [D, DFF]^T -> [NDS, P, KF, DSW]
            b = w2[e].T.reshape(KF, P, NDS, DSW)
            w2h[j] = (b.transpose(2, 1, 0, 3)
                      .reshape(NDS, P, KF * DSW).astype(BF16))
        in_maps.append({"xt": xt, "w01": w01, "w2": w2h})
    coef = [s2[e] * cw[e, idx[e]] for e in range(En)]
    return in_maps, idx, coef, slot_expert, TGs, CH, (Tn, Dn)


def _combine(results, idx, coef, slot_expert, TGs, CH, shapes):
    """Unshard: un-transpose each expert's y and scatter-add with s2*cw."""
    Tn, Dn = shapes
    TGM = max(TGs)
    out = np.zeros((Tn, Dn), np.float32)
    for c in range(NCORES):
        y = results[c]["y"]  # [V, NDS, P, NDP*TGM] bf16
        for j in range(EPC):
            e = slot_expert[c][j]
            TG = TGs[j]
            ids = idx[e]
            for ch in range(CH):
                sl = ids[ch * TG:(ch + 1) * TG]
                if not len(sl):
                    continue
                # [NDS, P, NDP, TG] -> [D, n] with d = ds*DSW + dj*P + p
                yv = (y[j * CH + ch][:, :, :NDP * TG]
                      .reshape(NDS, P, NDP, TG)[..., :len(sl)]
                      .astype(np.float32).transpose(0, 2, 1, 3)
                      .reshape(Dn, len(sl)))
                cf = coef[e][ch * TG:ch * TG + len(sl)]
                out[sl] += cf[:, None] * yv.T
    return out


def _ensure_axon_ntff_hook():
    """Provide antenv.axon_hooks if the image's antenv stub lacks it."""
    import sys
    import types
    try:
        import antenv.axon_hooks  # noqa: F401
        return
    except ImportError:
        pass
    try:
        import antenv

        mod = types.ModuleType("antenv.axon_hooks")
        _state = {"hook": None}
        mod.set_axon_ntff_profile_hook = lambda h: _state.__setitem__("hook", h)
        mod.get_axon_ntff_profile_hook = lambda: _state["hook"]
        sys.modules["antenv.axon_hooks"] = mod
        antenv.axon_hooks = mod
        try:
            from trn_agent_boot.trn_boot import _ntff_profile_via_ctypes

            so = "/opt/axon/libaxon_pjrt.so"
            if os.path.exists(so):
                mod.set_axon_ntff_profile_hook(_ntff_profile_via_ctypes(so))
        except Exception:
            pass
    except Exception:
        pass


def kernel(**inputs) -> np.ndarray:
    global LAST_RESULT
    _ensure_axon_ntff_hook()
    from concourse.bass_utils import run_bass_kernel_spmd

    in_maps, idx, coef, slot_expert, TGs, CH, shapes = _prep_host(inputs)

    key = (TGs, CH)
    nc = _PROGRAM_CACHE.get(key)
    if nc is None:
        nc = _build_program(TGs, CH)
        _PROGRAM_CACHE[key] = nc

    res = run_bass_kernel_spmd(nc, in_maps, core_ids=list(range(NCORES)))
    LAST_RESULT = res
    return _combine(res.results, idx, coef, slot_expert, TGs, CH, shapes)


# revision 18
# speedup vs baseline: 1.0011x; 1.0011x over previous
"""DeepSeek-style MoE block (SwiGLU experts, top-k routing) on 8 Trainium2 cores.

Expert-parallel: each of the 8 cores owns E/8 = 2 experts and receives only the
tokens routed to those experts (host-side dispatch).

Design (vs the fp32r baseline at ~223 us):
 - Everything streamed from HBM is bf16 (weights, gathered x, output y), which
   halves the dominant DMA traffic (56.6 MB -> ~31 MB per core) and enables the
   PE's Fast Weight Load path. Matmul rate is unchanged (1 row/cycle for bf16
   and fp32r alike); PSUM accumulates fp32. Measured end-to-end rel err of the
   bf16 pipeline is ~4.4e-3 (gate 2e-2).
 - s0/s1 are folded into w0/w1 on the host (both scales are linear
   pre-activation), s2*cw is folded into the host combine, so the device
   kernel is pure silu(x@w0T)*(x@w1T) @ w2T.
 - Tokens stay on the matmul free dim in BOTH phases (phase 2 is transposed:
   y^T = w2 @ h^T), so compute scales with the padded token count, not with
   128-tile granularity.
 - Per-slot token padding: experts are sorted by token count; the 8 largest go
   to slot 0 (TG0 = roundup8 of their max), the 8 smallest to slot 1 (TG1).
   The SPMD program bakes both shapes in, cutting ~5% of padded compute vs a
   single global TG.
 - Host pre-swizzles every DRAM operand into the exact SBUF layout so DMAs are
   large fully-contiguous transfers; expert 0's xt/w01 are chunked 4-way and
   pinned to opposite HWDGE rings so the first matmuls start ~6 us earlier.
 - Large DMAs alternate across the two HWDGE rings (nc.sync / nc.scalar); w2
   streams via SWDGE (gpsimd) so phase-2 weights don't queue behind the
   phase-1 weight stream on the rings.
"""

import os
import numpy as np
import ml_dtypes

T, D, DFF, E, TOPK = 1024, 2048, 1024, 16, 6
NCORES, P = 8, 128
EPC = E // NCORES   # experts per core
KD = D // P         # contraction tiles over D (phase 1)
KF = DFF // P       # contraction tiles over DFF (phase 2)
FG = 2              # f-tiles (128 rows of DFF) per PSUM group, phase 1
FGP = FG * P
NFG = KF // FG      # phase-1 weight groups per expert
DSW = 512           # phase-2 D-slice width
NDS = D // DSW      # phase-2 D slices
NDP = DSW // P      # d-tiles per D slice

BF16 = ml_dtypes.bfloat16

# Set by kernel() after each run: BassKernelResults (exec_time_ns when traced).
LAST_RESULT = None

_PROGRAM_CACHE = {}


def _build_program(TGs, CH):
    """SPMD single-core program. Slot shapes TGs (len EPC) are compile-time;
    CH > 1 token-chunks (uniform TG) only when some expert gets > 512 tokens;
    weights are re-streamed per chunk in that rare fallback."""
    import concourse.bacc as bacc
    import concourse.mybir as mybir
    import concourse.tile as tile

    assert all(tg <= 512 for tg in TGs)
    f32 = mybir.dt.float32
    bf16 = mybir.dt.bfloat16
    Silu = mybir.ActivationFunctionType.Silu

    V = EPC * CH
    TGM = max(TGs)
    nc = bacc.Bacc("TRN2", target_bir_lowering=False, debug=False)

    xt_d = nc.dram_tensor("xt", [V, P, KD * TGM], bf16,
                          kind="ExternalInput").ap()
    w01_d = nc.dram_tensor("w01", [EPC, NFG, P, KD * 2 * FGP], bf16,
                           kind="ExternalInput").ap()
    w2_d = nc.dram_tensor("w2", [EPC, NDS, P, KF * DSW], bf16,
                          kind="ExternalInput").ap()
    y_d = nc.dram_tensor("y", [V, NDS, P, NDP * TGM], bf16,
                         kind="ExternalOutput").ap()

    with tile.TileContext(nc) as tc:
        # Alternate large DMAs across the two physical HWDGE rings.
        rings = [nc.sync, nc.scalar]
        ring_state = [0]

        def ring():
            ring_state[0] ^= 1
            return rings[ring_state[0]]

        KCH = 4           # k-tiles per startup chunk
        NCHK = KD // KCH  # startup chunks for xt / first w01 group

        with (
            tc.tile_pool(name="xt", bufs=2) as xt_pool,
            tc.tile_pool(name="xt0", bufs=NCHK) as xt0_pool,
            tc.tile_pool(name="w01", bufs=5) as w01_pool,
            tc.tile_pool(name="w010", bufs=NCHK) as w010_pool,
            tc.tile_pool(name="w2", bufs=3) as w2_pool,
            tc.tile_pool(name="ht", bufs=2) as ht_pool,
            tc.tile_pool(name="y", bufs=3) as y_pool,
            tc.tile_pool(name="ydj", bufs=4) as ydj_pool,
            tc.tile_pool(name="sig", bufs=4) as sig_pool,
            tc.tile_pool(name="pgu", bufs=6, space="PSUM") as pgu_pool,
            tc.tile_pool(name="py", bufs=2, space="PSUM") as py_pool,
        ):
            for v in range(V):
                e = v // CH
                TG = TGs[e]
                # The PE consumes w01 at ~215 GB/s -- about one full HWDGE
                # ring -- so w01 gets the sync ring to itself; xt and y share
                # the scalar ring; w2 streams via SWDGE. Expert 0's xt and
                # first weight group are 4-way chunked so the k=0..3 matmuls
                # start after ~0.9 MB instead of the whole 3.5 MB prefix.
                if v == 0:
                    xtc = []
                    w01c = []
                    for c in range(NCHK):
                        t = xt0_pool.tile([P, KCH * TG], bf16, tag="xt0")
                        nc.scalar.dma_start(
                            t[:],
                            xt_d[v][:, c * KCH * TG:(c + 1) * KCH * TG])
                        xtc.append(t)
                        t = w010_pool.tile([P, KCH, 2, FGP], bf16, tag="w010")
                        nc.sync.dma_start(
                            t[:],
                            w01_d[e, 0][:, c * KCH * 2 * FGP:
                                        (c + 1) * KCH * 2 * FGP]
                            .rearrange("p (k g c) -> p k g c", k=KCH, g=2))
                        w01c.append(t)

                    def xt_ap(k):
                        return xtc[k // KCH][:, (k % KCH) * TG:
                                             (k % KCH + 1) * TG]
                else:
                    xt = xt_pool.tile([P, KD * TGM], bf16, tag="xt")
                    nc.scalar.dma_start(xt[:, :KD * TG], xt_d[v][:, :KD * TG])

                    def xt_ap(k):
                        return xt[:, k * TG:(k + 1) * TG]

                # --- phase 1: ht = silu(x@w0T)*(x@w1T), [DFF, TG] bf16 ---
                ht = ht_pool.tile([P, KF * TGM], bf16, tag="ht")
                for fg in range(NFG):
                    if v == 0 and fg == 0:
                        # Chunk tiles were loaded above, paired with xt.
                        def w01_ap(k, gu, j):
                            return w01c[k // KCH][:, k % KCH, gu,
                                                  j * P:(j + 1) * P]
                    else:
                        w01b = w01_pool.tile([P, KD, 2, FGP], bf16, tag="w01b")
                        nc.sync.dma_start(
                            w01b[:],
                            w01_d[e, fg].rearrange("p (k g c) -> p k g c",
                                                   k=KD, g=2))

                        def w01_ap(k, gu, j):
                            return w01b[:, k, gu, j * P:(j + 1) * P]
                    psG = [pgu_pool.tile([P, TGM], f32, tag="pgu",
                                         name=f"psG_{v}_{fg}_{j}")
                           for j in range(FG)]
                    psU = [pgu_pool.tile([P, TGM], f32, tag="pgu",
                                         name=f"psU_{v}_{fg}_{j}")
                           for j in range(FG)]
                    for k in range(KD):
                        for j in range(FG):
                            nc.tensor.matmul(
                                psG[j][:, :TG], w01_ap(k, 0, j), xt_ap(k),
                                start=(k == 0), stop=(k == KD - 1))
                            nc.tensor.matmul(
                                psU[j][:, :TG], w01_ap(k, 1, j), xt_ap(k),
                                start=(k == 0), stop=(k == KD - 1))
                    for j in range(FG):
                        f = fg * FG + j
                        sig = sig_pool.tile([P, TGM], f32, tag="sig")
                        nc.scalar.activation(sig[:, :TG], psG[j][:, :TG], Silu)
                        nc.vector.tensor_mul(
                            ht[:, f * TG:(f + 1) * TG], sig[:, :TG],
                            psU[j][:, :TG])

                # --- phase 2: y^T = (h @ w2T)^T = w2 @ h^T, [D, TG] bf16 ---
                for ds in range(NDS):
                    w2b = w2_pool.tile([P, KF, DSW], bf16, tag="w2b")
                    nc.gpsimd.dma_start(
                        w2b[:], w2_d[e, ds].rearrange("p (k c) -> p k c", k=KF))
                    last = (v == V - 1 and ds == NDS - 1)
                    yds = None if last else y_pool.tile([P, NDP * TGM], bf16,
                                                        tag="yds")
                    for dj in range(NDP):
                        psY = py_pool.tile([P, TGM], f32, tag="py",
                                           name=f"psY_{v}_{ds}_{dj}")
                        for k in range(KF):
                            nc.tensor.matmul(
                                psY[:, :TG], w2b[:, k, dj * P:(dj + 1) * P],
                                ht[:, k * TG:(k + 1) * TG],
                                start=(k == 0), stop=(k == KF - 1))
                        if last:
                            # Drain the final D-slice per d-tile so the last
                            # output DMA isn't one big post-compute transfer.
                            yj = ydj_pool.tile([P, TGM], bf16, tag="ydj")
                            if dj % 2 == 0:
                                nc.scalar.copy(yj[:, :TG], psY[:, :TG])
                            else:
                                nc.vector.tensor_copy(yj[:, :TG], psY[:, :TG])
                            nc.scalar.dma_start(
                                y_d[v, ds][:, dj * TG:(dj + 1) * TG],
                                yj[:, :TG])
                        elif dj % 2 == 0:
                            nc.scalar.copy(yds[:, dj * TG:(dj + 1) * TG],
                                           psY[:, :TG])
                        else:
                            nc.vector.tensor_copy(
                                yds[:, dj * TG:(dj + 1) * TG], psY[:, :TG])
                    if not last:
                        nc.scalar.dma_start(y_d[v, ds][:, :NDP * TG],
                                            yds[:, :NDP * TG])

    nc.compile()
    return nc


def _roundup8(n):
    return max(64, -(-int(n) // 8) * 8)


def _prep_host(inputs):
    """Host-side dispatch: routing weights, expert->slot assignment by token
    count, per-expert token gather, bf16 quantization, and pre-swizzle of
    every operand into its SBUF layout."""
    x = np.asarray(inputs["x"], dtype=np.float32)
    w0 = np.asarray(inputs["w0"], dtype=np.float32)
    w1 = np.asarray(inputs["w1"], dtype=np.float32)
    w2 = np.asarray(inputs["w2"], dtype=np.float32)
    s0 = np.asarray(inputs["s0"], dtype=np.float32)
    s1 = np.asarray(inputs["s1"], dtype=np.float32)
    s2 = np.asarray(inputs["s2"], dtype=np.float32)
    se = np.asarray(inputs["selected_experts"]).astype(np.int64)
    rw = np.asarray(inputs["routing_weights"], dtype=np.float32)

    Tn, Dn = x.shape
    En = w0.shape[0]
    assert (Dn, w0.shape[1], En) == (D, DFF, E)

    # combine weight per (expert, token): sum of routing weights over top-k
    cw = np.zeros((En, Tn), np.float32)
    cols = np.arange(Tn)
    for k in range(se.shape[1]):
        np.add.at(cw, (se[:, k], cols), rw[:, k])

    idx = [np.flatnonzero(cw[e] != 0.0) for e in range(En)]
    counts = np.array([len(i) for i in idx])
    maxn = int(counts.max())
    if maxn > 512:
        CH = -(-maxn // 512)
        tg = _roundup8(-(-maxn // CH))
        TGs = (tg,) * EPC
        order = np.arange(En)
    else:
        CH = 1
        # largest experts -> slot 0, smallest -> slot 1
        order = np.argsort(-counts, kind="stable")
        TGs = tuple(_roundup8(counts[order[s * NCORES:(s + 1) * NCORES]].max())
                    for s in range(EPC))
    TGM = max(TGs)

    # slot_expert[c][s] = expert id handled by core c, slot s
    slot_expert = [[int(order[s * NCORES + c]) for s in range(EPC)]
                   for c in range(NCORES)]

    xT = np.ascontiguousarray(x.T).astype(BF16)  # [D, T] bf16
    in_maps = []
    for c in range(NCORES):
        xt = np.zeros((EPC * CH, P, KD * TGM), BF16)
        w01 = np.empty((EPC, NFG, P, KD * 2 * FGP), BF16)
        w2h = np.empty((EPC, NDS, P, KF * DSW), BF16)
        for j in range(EPC):
            e = slot_expert[c][j]
            TG = TGs[j]
            ids = idx[e]
            for ch in range(CH):
                sl = ids[ch * TG:(ch + 1) * TG]
                g = np.zeros((KD, P, TG), BF16)
                g[:, :, :len(sl)] = xT[:, sl].reshape(KD, P, len(sl))
                xt[j * CH + ch, :, :KD * TG] = (
                    g.transpose(1, 0, 2).reshape(P, KD * TG))
            # [2, D, DFF] -> [NFG, P, KD, 2, FGP], scales folded in
            wt = np.stack([(w0[e] * s0[e]).T, (w1[e] * s1[e]).T], 0)
            a = wt.reshape(2, KD, P, NFG, FGP)
            w01[j] = (a.transpose(3, 2, 1, 0, 4)
                      .reshape(NFG, P, KD * 2 * FGP).astype(BF16))
            # [D, DFF]^T -> [NDS, P, KF, DSW]
            b = w2[e].T.reshape(KF, P, NDS, DSW)
            w2h[j] = (b.transpose(2, 1, 0, 3)
                      .reshape(NDS, P, KF * DSW).astype(BF16))
        in_maps.append({"xt": xt, "w01": w01, "w2": w2h})
    coef = [s2[e] * cw[e, idx[e]] for e in range(En)]
    return in_maps, idx, coef, slot_expert, TGs, CH, (Tn, Dn)


def _combine(results, idx, coef, slot_expert, TGs, CH, shapes):
    """Unshard: un-transpose each expert's y and scatter-add with s2*cw."""
    Tn, Dn = shapes
    TGM = max(TGs)
    out = np.zeros((Tn, Dn), np.float32)
    for c in range(NCORES):
        y = results[c]["y"]  # [V, NDS, P, NDP*TGM] bf16
        for j in range(EPC):
            e = slot_expert[c][j]
            TG = TGs[j]
            ids = idx[e]
            for ch in range(CH):
                sl = ids[ch * TG:(ch + 1) * TG]
                if not len(sl):
                    continue
                # [NDS, P, NDP, TG] -> [D, n] with d = ds*DSW + dj*P + p
                yv = (y[j * CH + ch][:, :, :NDP * TG]
                      .reshape(NDS, P, NDP, TG)[..., :len(sl)]
                      .astype(np.float32).transpose(0, 2, 1, 3)
                      .reshape(Dn, len(sl)))
                cf = coef[e][ch * TG:ch * TG + len(sl)]
                out[sl] += cf[:, None] * yv.T
    return out


def _ensure_axon_ntff_hook():
    """Provide antenv.axon_hooks if the image's antenv stub lacks it."""
    import sys
    import types
    try:
        import antenv.axon_hooks  # noqa: F401
        return
    except ImportError:
        pass
    try:
        import antenv

        mod = types.ModuleType("antenv.axon_hooks")
        _state = {"hook": None}
        mod.set_axon_ntff_profile_hook = lambda h: _state.__setitem__("hook", h)
        mod.get_axon_ntff_profile_hook = lambda: _state["hook"]
        sys.modules["antenv.axon_hooks"] = mod
        antenv.axon_hooks = mod
        try:
            from trn_agent_boot.trn_boot import _ntff_profile_via_ctypes

            so = "/opt/axon/libaxon_pjrt.so"
            if os.path.exists(so):
                mod.set_axon_ntff_profile_hook(_ntff_profile_via_ctypes(so))
        except Exception:
            pass
    except Exception:
        pass


def kernel(**inputs) -> np.ndarray:
    global LAST_RESULT
    _ensure_axon_ntff_hook()
    from concourse.bass_utils import run_bass_kernel_spmd

    in_maps, idx, coef, slot_expert, TGs, CH, shapes = _prep_host(inputs)

    key = (TGs, CH)
    nc = _PROGRAM_CACHE.get(key)
    if nc is None:
        nc = _build_program(TGs, CH)
        _PROGRAM_CACHE[key] = nc

    res = run_bass_kernel_spmd(nc, in_maps, core_ids=list(range(NCORES)))
    LAST_RESULT = res
    return _combine(res.results, idx, coef, slot_expert, TGs, CH, shapes)


# revision 25
# speedup vs baseline: 1.0897x; 1.0885x over previous
"""DeepSeek-style MoE block (SwiGLU experts, top-k routing) on 8 Trainium2 cores.

Expert-parallel: each of the 8 cores owns E/8 = 2 experts and receives only the
tokens routed to those experts (host-side dispatch).

Design (vs the fp32r baseline at ~223 us):
 - Everything streamed from HBM is bf16 (weights, gathered x, output y), which
   halves the dominant DMA traffic (56.6 MB -> ~31 MB per core) and enables the
   PE's Fast Weight Load path. Matmul rate is unchanged (1 row/cycle for bf16
   and fp32r alike); PSUM accumulates fp32. Measured end-to-end rel err of the
   bf16 pipeline is ~4.4e-3 (gate 2e-2).
 - s0/s1 are folded into w0/w1 on the host (both scales are linear
   pre-activation), s2*cw is folded into the host combine, so the device
   kernel is pure silu(x@w0T)*(x@w1T) @ w2T.
 - Tokens stay on the matmul free dim in BOTH phases (phase 2 is transposed:
   y^T = w2 @ h^T), so compute scales with the padded token count, not with
   128-tile granularity.
 - Per-slot token padding: experts are sorted by token count; the 8 largest go
   to slot 0 (TG0 = roundup8 of their max), the 8 smallest to slot 1 (TG1).
   The SPMD program bakes both shapes in, cutting ~5% of padded compute vs a
   single global TG.
 - Host pre-swizzles every DRAM operand into the exact SBUF layout so DMAs are
   large fully-contiguous transfers; expert 0's xt/w01 are chunked 4-way and
   pinned to opposite HWDGE rings so the first matmuls start ~6 us earlier.
 - Large DMAs alternate across the two HWDGE rings (nc.sync / nc.scalar); w2
   streams via SWDGE (gpsimd) so phase-2 weights don't queue behind the
   phase-1 weight stream on the rings.
"""

import os
import numpy as np
import ml_dtypes

T, D, DFF, E, TOPK = 1024, 2048, 1024, 16, 6
NCORES, P = 8, 128
EPC = E // NCORES   # experts per core
KD = D // P         # contraction tiles over D (phase 1)
KF = DFF // P       # contraction tiles over DFF (phase 2)
FG = 2              # f-tiles (128 rows of DFF) per PSUM group, phase 1
FGP = FG * P
NFG = KF // FG      # phase-1 weight groups per expert
DSW = 512           # phase-2 D-slice width
NDS = D // DSW      # phase-2 D slices
NDP = DSW // P      # d-tiles per D slice

BF16 = ml_dtypes.bfloat16

# Set by kernel() after each run: BassKernelResults (exec_time_ns when traced).
LAST_RESULT = None

_PROGRAM_CACHE = {}


def _build_program(TGs, CH):
    """SPMD single-core program. Slot shapes TGs (len EPC) are compile-time;
    CH > 1 token-chunks (uniform TG) only when some expert gets > 512 tokens;
    weights are re-streamed per chunk in that rare fallback."""
    import concourse.bacc as bacc
    import concourse.mybir as mybir
    import concourse.tile as tile

    assert all(tg <= 512 for tg in TGs)
    f32 = mybir.dt.float32
    bf16 = mybir.dt.bfloat16
    Silu = mybir.ActivationFunctionType.Silu

    V = EPC * CH
    TGM = max(TGs)
    nc = bacc.Bacc("TRN2", target_bir_lowering=False, debug=False)

    xt_d = nc.dram_tensor("xt", [V, P, KD * TGM], bf16,
                          kind="ExternalInput").ap()
    w01_d = nc.dram_tensor("w01", [EPC, NFG, P, KD * 2 * FGP], bf16,
                           kind="ExternalInput").ap()
    w2_d = nc.dram_tensor("w2", [EPC, NDS, P, KF * DSW], bf16,
                          kind="ExternalInput").ap()
    y_d = nc.dram_tensor("y", [V, NDS, P, NDP * TGM], bf16,
                         kind="ExternalOutput").ap()

    with tile.TileContext(nc) as tc:
        # Alternate large DMAs across the two physical HWDGE rings.
        rings = [nc.sync, nc.scalar]
        ring_state = [0]

        def ring():
            ring_state[0] ^= 1
            return rings[ring_state[0]]

        KCH = 4           # k-tiles per startup chunk
        NCHK = KD // KCH  # startup chunks for xt / first w01 group

        with (
            tc.tile_pool(name="xt", bufs=2) as xt_pool,
            tc.tile_pool(name="xt0", bufs=NCHK) as xt0_pool,
            tc.tile_pool(name="w01", bufs=6) as w01_pool,
            tc.tile_pool(name="w010", bufs=NCHK) as w010_pool,
            tc.tile_pool(name="ht", bufs=2) as ht_pool,
            tc.tile_pool(name="y", bufs=3) as y_pool,
            tc.tile_pool(name="ydj", bufs=4) as ydj_pool,
            tc.tile_pool(name="sig", bufs=4) as sig_pool,
            tc.tile_pool(name="pgu", bufs=6, space="PSUM") as pgu_pool,
            tc.tile_pool(name="py", bufs=2, space="PSUM") as py_pool,
        ):
            for v in range(V):
                e = v // CH
                TG = TGs[e]
                # The PE consumes w01 at ~215 GB/s -- about one full HWDGE
                # ring -- so w01 gets the sync ring to itself; xt and y share
                # the scalar ring; w2 streams via SWDGE. Expert 0's xt and
                # first weight group are 4-way chunked so the k=0..3 matmuls
                # start after ~0.9 MB instead of the whole 3.5 MB prefix.
                if v == 0:
                    xtc = []
                    w01c = []
                    for c in range(NCHK):
                        t = xt0_pool.tile([P, KCH * TG], bf16, tag="xt0")
                        nc.scalar.dma_start(
                            t[:],
                            xt_d[v][:, c * KCH * TG:(c + 1) * KCH * TG])
                        xtc.append(t)
                        t = w010_pool.tile([P, KCH, 2, FGP], bf16, tag="w010")
                        nc.sync.dma_start(
                            t[:],
                            w01_d[e, 0][:, c * KCH * 2 * FGP:
                                        (c + 1) * KCH * 2 * FGP]
                            .rearrange("p (k g c) -> p k g c", k=KCH, g=2))
                        w01c.append(t)

                    def xt_ap(k):
                        return xtc[k // KCH][:, (k % KCH) * TG:
                                             (k % KCH + 1) * TG]
                else:
                    xt = xt_pool.tile([P, KD * TGM], bf16, tag="xt")
                    ring().dma_start(xt[:, :KD * TG], xt_d[v][:, :KD * TG])

                    def xt_ap(k):
                        return xt[:, k * TG:(k + 1) * TG]

                # --- phase 1: ht = silu(x@w0T)*(x@w1T), [DFF, TG] bf16 ---
                ht = ht_pool.tile([P, KF * TGM], bf16, tag="ht")
                for fg in range(NFG):
                    if v == 0 and fg == 0:
                        # Chunk tiles were loaded above, paired with xt.
                        def w01_ap(k, gu, j):
                            return w01c[k // KCH][:, k % KCH, gu,
                                                  j * P:(j + 1) * P]
                    else:
                        w01b = w01_pool.tile([P, KD, 2, FGP], bf16, tag="w01b")
                        ring().dma_start(
                            w01b[:],
                            w01_d[e, fg].rearrange("p (k g c) -> p k g c",
                                                   k=KD, g=2))

                        def w01_ap(k, gu, j):
                            return w01b[:, k, gu, j * P:(j + 1) * P]
                    psG = [pgu_pool.tile([P, TGM], f32, tag="pgu",
                                         name=f"psG_{v}_{fg}_{j}")
                           for j in range(FG)]
                    psU = [pgu_pool.tile([P, TGM], f32, tag="pgu",
                                         name=f"psU_{v}_{fg}_{j}")
                           for j in range(FG)]
                    for k in range(KD):
                        for j in range(FG):
                            nc.tensor.matmul(
                                psG[j][:, :TG], w01_ap(k, 0, j), xt_ap(k),
                                start=(k == 0), stop=(k == KD - 1))
                            nc.tensor.matmul(
                                psU[j][:, :TG], w01_ap(k, 1, j), xt_ap(k),
                                start=(k == 0), stop=(k == KD - 1))
                    for j in range(FG):
                        f = fg * FG + j
                        sig = sig_pool.tile([P, TGM], f32, tag="sig")
                        nc.scalar.activation(sig[:, :TG], psG[j][:, :TG], Silu)
                        nc.vector.tensor_mul(
                            ht[:, f * TG:(f + 1) * TG], sig[:, :TG],
                            psU[j][:, :TG])

                # --- phase 2: y^T = (h @ w2T)^T = w2 @ h^T, [D, TG] bf16 ---
                # w2 shares the w01 pool (same tile size, half filled): the
                # weight streams form one FIFO per ring in exact consumption
                # order, so phase-2 weights never race the phase-1 stream.
                for ds in range(NDS):
                    w2b = w01_pool.tile([P, KD, 2, FGP], bf16, tag="w01b")
                    ring().dma_start(
                        w2b[:, :KF, :, :],
                        w2_d[e, ds].rearrange("p (k g c) -> p k g c",
                                              k=KF, g=2))
                    last = (v == V - 1 and ds == NDS - 1)
                    yds = None if last else y_pool.tile([P, NDP * TGM], bf16,
                                                        tag="yds")
                    for dj in range(NDP):
                        psY = py_pool.tile([P, TGM], f32, tag="py",
                                           name=f"psY_{v}_{ds}_{dj}")
                        for k in range(KF):
                            nc.tensor.matmul(
                                psY[:, :TG],
                                w2b[:, k, dj // 2,
                                    (dj % 2) * P:(dj % 2 + 1) * P],
                                ht[:, k * TG:(k + 1) * TG],
                                start=(k == 0), stop=(k == KF - 1))
                        if last:
                            # Drain the final D-slice per d-tile so the last
                            # output DMA isn't one big post-compute transfer.
                            yj = ydj_pool.tile([P, TGM], bf16, tag="ydj")
                            if dj % 2 == 0:
                                nc.scalar.copy(yj[:, :TG], psY[:, :TG])
                            else:
                                nc.vector.tensor_copy(yj[:, :TG], psY[:, :TG])
                            ring().dma_start(
                                y_d[v, ds][:, dj * TG:(dj + 1) * TG],
                                yj[:, :TG])
                        elif dj % 2 == 0:
                            nc.scalar.copy(yds[:, dj * TG:(dj + 1) * TG],
                                           psY[:, :TG])
                        else:
                            nc.vector.tensor_copy(
                                yds[:, dj * TG:(dj + 1) * TG], psY[:, :TG])
                    if not last:
                        ring().dma_start(y_d[v, ds][:, :NDP * TG],
                                         yds[:, :NDP * TG])

    nc.compile()
    return nc


def _roundup8(n):
    return max(64, -(-int(n) // 8) * 8)


def _prep_host(inputs):
    """Host-side dispatch: routing weights, expert->slot assignment by token
    count, per-expert token gather, bf16 quantization, and pre-swizzle of
    every operand into its SBUF layout."""
    x = np.asarray(inputs["x"], dtype=np.float32)
    w0 = np.asarray(inputs["w0"], dtype=np.float32)
    w1 = np.asarray(inputs["w1"], dtype=np.float32)
    w2 = np.asarray(inputs["w2"], dtype=np.float32)
    s0 = np.asarray(inputs["s0"], dtype=np.float32)
    s1 = np.asarray(inputs["s1"], dtype=np.float32)
    s2 = np.asarray(inputs["s2"], dtype=np.float32)
    se = np.asarray(inputs["selected_experts"]).astype(np.int64)
    rw = np.asarray(inputs["routing_weights"], dtype=np.float32)

    Tn, Dn = x.shape
    En = w0.shape[0]
    assert (Dn, w0.shape[1], En) == (D, DFF, E)

    # combine weight per (expert, token): sum of routing weights over top-k
    cw = np.zeros((En, Tn), np.float32)
    cols = np.arange(Tn)
    for k in range(se.shape[1]):
        np.add.at(cw, (se[:, k], cols), rw[:, k])

    idx = [np.flatnonzero(cw[e] != 0.0) for e in range(En)]
    counts = np.array([len(i) for i in idx])
    maxn = int(counts.max())
    if maxn > 512:
        CH = -(-maxn // 512)
        tg = _roundup8(-(-maxn // CH))
        TGs = (tg,) * EPC
        order = np.arange(En)
    else:
        CH = 1
        # largest experts -> slot 0, smallest -> slot 1
        order = np.argsort(-counts, kind="stable")
        TGs = tuple(_roundup8(counts[order[s * NCORES:(s + 1) * NCORES]].max())
                    for s in range(EPC))
    TGM = max(TGs)

    # slot_expert[c][s] = expert id handled by core c, slot s
    slot_expert = [[int(order[s * NCORES + c]) for s in range(EPC)]
                   for c in range(NCORES)]

    xT = np.ascontiguousarray(x.T).astype(BF16)  # [D, T] bf16
    in_maps = []
    for c in range(NCORES):
        xt = np.zeros((EPC * CH, P, KD * TGM), BF16)
        w01 = np.empty((EPC, NFG, P, KD * 2 * FGP), BF16)
        w2h = np.empty((EPC, NDS, P, KF * DSW), BF16)
        for j in range(EPC):
            e = slot_expert[c][j]
            TG = TGs[j]
            ids = idx[e]
            for ch in range(CH):
                sl = ids[ch * TG:(ch + 1) * TG]
                g = np.zeros((KD, P, TG), BF16)
                g[:, :, :len(sl)] = xT[:, sl].reshape(KD, P, len(sl))
                xt[j * CH + ch, :, :KD * TG] = (
                    g.transpose(1, 0, 2).reshape(P, KD * TG))
            # [2, D, DFF] -> [NFG, P, KD, 2, FGP], scales folded in
            wt = np.stack([(w0[e] * s0[e]).T, (w1[e] * s1[e]).T], 0)
            a = wt.reshape(2, KD, P, NFG, FGP)
            w01[j] = (a.transpose(3, 2, 1, 0, 4)
                      .reshape(NFG, P, KD * 2 * FGP).astype(BF16))
            # [D, DFF]^T -> [NDS, P, KF, DSW]
            b = w2[e].T.reshape(KF, P, NDS, DSW)
            w2h[j] = (b.transpose(2, 1, 0, 3)
                      .reshape(NDS, P, KF * DSW).astype(BF16))
        in_maps.append({"xt": xt, "w01": w01, "w2": w2h})
    coef = [s2[e] * cw[e, idx[e]] for e in range(En)]
    return in_maps, idx, coef, slot_expert, TGs, CH, (Tn, Dn)


def _combine(results, idx, coef, slot_expert, TGs, CH, shapes):
    """Unshard: un-transpose each expert's y and scatter-add with s2*cw."""
    Tn, Dn = shapes
    TGM = max(TGs)
    out = np.zeros((Tn, Dn), np.float32)
    for c in range(NCORES):
        y = results[c]["y"]  # [V, NDS, P, NDP*TGM] bf16
        for j in range(EPC):
            e = slot_expert[c][j]
            TG = TGs[j]
            ids = idx[e]
            for ch in range(CH):
                sl = ids[ch * TG:(ch + 1) * TG]
                if not len(sl):
                    continue
                # [NDS, P, NDP, TG] -> [D, n] with d = ds*DSW + dj*P + p
                yv = (y[j * CH + ch][:, :, :NDP * TG]
                      .reshape(NDS, P, NDP, TG)[..., :len(sl)]
                      .astype(np.float32).transpose(0, 2, 1, 3)
                      .reshape(Dn, len(sl)))
                cf = coef[e][ch * TG:ch * TG + len(sl)]
                out[sl] += cf[:, None] * yv.T
    return out


def _ensure_axon_ntff_hook():
    """Provide antenv.axon_hooks if the image's antenv stub lacks it."""
    import sys
    import types
    try:
        import antenv.axon_hooks  # noqa: F401
        return
    except ImportError:
        pass
    try:
        import antenv

        mod = types.ModuleType("antenv.axon_hooks")
        _state = {"hook": None}
        mod.set_axon_ntff_profile_hook = lambda h: _state.__setitem__("hook", h)
        mod.get_axon_ntff_profile_hook = lambda: _state["hook"]
        sys.modules["antenv.axon_hooks"] = mod
        antenv.axon_hooks = mod
        try:
            from trn_agent_boot.trn_boot import _ntff_profile_via_ctypes

            so = "/opt/axon/libaxon_pjrt.so"
            if os.path.exists(so):
                mod.set_axon_ntff_profile_hook(_ntff_profile_via_ctypes(so))
        except Exception:
            pass
    except Exception:
        pass


def kernel(**inputs) -> np.ndarray:
    global LAST_RESULT
    _ensure_axon_ntff_hook()
    from concourse.bass_utils import run_bass_kernel_spmd

    in_maps, idx, coef, slot_expert, TGs, CH, shapes = _prep_host(inputs)

    key = (TGs, CH)
    nc = _PROGRAM_CACHE.get(key)
    if nc is None:
        nc = _build_program(TGs, CH)
        _PROGRAM_CACHE[key] = nc

    res = run_bass_kernel_spmd(nc, in_maps, core_ids=list(range(NCORES)))
    LAST_RESULT = res
    return _combine(res.results, idx, coef, slot_expert, TGs, CH, shapes)
